# revision 23
# baseline (speedup 1.0000x reference)
"""Position-attention kernel for Trainium2 (8 NeuronCores, SPMD data-parallel).

Math (per batch b):
    q = X Wq ; k = X Wk ; v = X Wv          (X = x[b] reshaped [N, C], N=4096, C=128)
    energy[i, j] = k_i . q_j
    attn = softmax(energy, axis=-1)
    out = gamma * (attn @ v) + X

Kernel restructuring:
    energy = X A X^T with A = Wq Wk^T, computed transposed as
    eT[j, i] = sum_c xT[c, j] * w[c, i]  where  w = A @ X_i^T  (host prep).
    eT lands in PSUM with j on partitions; softmax is shift/scale invariant so
    probabilities are computed as p~ = exp(e - SHIFT) * 2^KSC, split across TWO
    engines:
      - scalar engine: exact exp via activation (bias = -SHIFT + KSC*ln2)
      - vector engine: Schraudolph bit-trick exp -- one tensor_scalar that
        writes bf16 BITS as uint16: bits = e*128*log2(e) + B16 (fp32->uint16
        conversion saturates on HW; negative bits only occur for dead cells
        whose relative weight is < e^-60). The uint16 tile is bitcast to bf16
        when used as the attn@v stationary.
    A ones-column appended to v gives the softmax denominator for free; the
    2^KSC scale cancels in the normalization. gamma is folded into v on the
    host (numerator scales, denominator doesn't), so the blend is just
    out = oa * recip(den) + x. w and v are tiny O(N C^2) weight preps and are
    computed on the host; the O(N^2) attention runs on device.

Sharding: 8 cores = (4 batches) x (2 halves of the 4096 output rows).
"""

import numpy as np

B, Dd, Hh, Ww, C = 4, 16, 16, 16, 128
N = Dd * Hh * Ww            # 4096 sequence positions (j)
NCORES = 8
NI = (B * N) // NCORES      # 2048 output rows per core (i)
NJB = N // 128              # 32 j-blocks
IC = 512                    # i-chunk width
NIC = NI // IC              # 4 i-chunks
NIT = IC // 128             # 4 i-tiles per chunk
G = 2                       # j-blocks per exp group (PSUM: 2*2 + 4 oa = 8 banks)
NG = NJB // G               # 16 groups
GROUPS = [(g * G, G) for g in range(NG)]
DVE_GROUPS = (0, 2, 4, 7, 9, 11, 14)  # interleaved so Act/DVE exps overlap

SHIFT = 32.0
KSC = 40.0                  # probabilities carry a 2^KSC scale (cancels)
LOG2E = 1.4426950408889634
LN2 = 0.6931471805599453
ACT_BIAS = -SHIFT + KSC * LN2                       # -4.274112...
A16 = 128.0 * LOG2E                                 # 184.66505...
B16 = 128.0 * (127.0 + KSC - SHIFT * LOG2E) - 7.0   # 15459.72...

OA_TAGS = ("pa", "pb", "pc", "pd")

_NC_CACHE = {}


def _build_nc():
    from contextlib import ExitStack

    import concourse.bacc as bacc
    import concourse.mybir as mybir
    import concourse.tile as tile

    dt = mybir.dt
    nc = bacc.Bacc(target_bir_lowering=False)

    xT_d = nc.declare_dram_parameter("xT", [128, N], dt.float16, isOutput=False)
    w_d = nc.declare_dram_parameter("w", [NIC, 128, 512], dt.float16, isOutput=False)
    v_d = nc.declare_dram_parameter(
        "v", [N // 512, 128, 4 * 132], dt.bfloat16, isOutput=False
    )
    xres_d = nc.declare_dram_parameter(
        "xres", [NIC, 128, NIT * 128], dt.bfloat16, isOutput=False
    )
    out_d = nc.declare_dram_parameter(
        "out", [NI // 128, 128, 128], dt.bfloat16, isOutput=True
    )

    NCH = N // 512  # 8 column chunks of xT
    with tile.TileContext(nc) as tc, ExitStack() as ctx:
        persist = ctx.enter_context(tc.tile_pool(name="persist", bufs=1))

        # warm up the exp table load while DMAs run
        dummy = persist.tile([1, 1], dt.float32)
        nc.vector.memset(dummy[:], 0.0)
        nc.scalar.activation(
            out=dummy[:], in_=dummy[:], func=mybir.ActivationFunctionType.Exp
        )
        # zeroed operand for PE-warmup matmuls
        warm = persist.tile([128, 128], dt.float16)
        nc.gpsimd.memset(warm[:], 0.0)

        xt_ch = [
            persist.tile([128, 512], dt.float16, name=f"xt{jc}") for jc in range(NCH)
        ]
        v_ch = [
            persist.tile([128, 4, 132], dt.bfloat16, name=f"v{jc}")
            for jc in range(NCH)
        ]
        w_ch = [persist.tile([128, 512], dt.float16, name=f"w{k}") for k in range(NIC)]
        xr_ch = [
            persist.tile([128, NIT, 128], dt.bfloat16, name=f"xr{ic}")
            for ic in range(NIC)
        ]

        # DMA order mirrors the critical path; two bulk queues (sync + gpsimd)
        nc.sync.dma_start(out=w_ch[0][:, 0:256], in_=w_d[0][:, 0:256])
        nc.gpsimd.dma_start(out=w_ch[0][:, 256:512], in_=w_d[0][:, 256:512])
        nc.sync.dma_start(out=xt_ch[0][:, 0:256], in_=xT_d[:, 0:256])
        nc.gpsimd.dma_start(out=xt_ch[0][:, 256:512], in_=xT_d[:, 256:512])
        nc.sync.dma_start(out=xt_ch[1][:, 0:256], in_=xT_d[:, 512:768])
        nc.gpsimd.dma_start(out=xt_ch[1][:, 256:512], in_=xT_d[:, 768:1024])
        nc.sync.dma_start(out=xt_ch[2][:], in_=xT_d[:, 1024:1536])
        nc.gpsimd.dma_start(out=xt_ch[3][:], in_=xT_d[:, 1536:2048])
        nc.sync.dma_start(out=xt_ch[4][:], in_=xT_d[:, 2048:2560])
        nc.gpsimd.dma_start(out=xt_ch[5][:], in_=xT_d[:, 2560:3072])
        nc.sync.dma_start(out=xt_ch[6][:], in_=xT_d[:, 3072:3584])
        nc.gpsimd.dma_start(out=v_ch[0][:], in_=v_d[0])
        nc.gpsimd.dma_start(out=xt_ch[7][:], in_=xT_d[:, 3584:4096])
        for jc in range(1, NCH):
            nc.gpsimd.dma_start(out=v_ch[jc][:], in_=v_d[jc])
        nc.sync.dma_start(out=w_ch[2][:], in_=w_d[2])
        nc.gpsimd.dma_start(out=w_ch[1][:], in_=w_d[1])
        nc.gpsimd.dma_start(out=w_ch[3][:], in_=w_d[3])
        for ic in range(NIC):
            nc.gpsimd.dma_start(out=xr_ch[ic][:], in_=xres_d[ic])

        shiftb = persist.tile([128, 1], dt.float32)
        nc.vector.memset(shiftb[:], ACT_BIAS)

        epool = ctx.enter_context(tc.tile_pool(name="epsum", bufs=2, space="PSUM"))
        opool = ctx.enter_context(tc.tile_pool(name="opsum", bufs=1, space="PSUM"))
        ptpool = ctx.enter_context(tc.tile_pool(name="ptp", bufs=6))
        pt16pool = ctx.enter_context(tc.tile_pool(name="pt16p", bufs=6))
        spool = ctx.enter_context(tc.tile_pool(name="small", bufs=4))
        osb_pool = ctx.enter_context(tc.tile_pool(name="osb", bufs=2))
        outpool = ctx.enter_context(tc.tile_pool(name="outp", bufs=4))

        # ---- PE warmup (p-state ramp while DMAs land) ----
        wpsum = opool.tile([128, 512], dt.float32, tag="pa", name="warmp")
        for r in range(8):
            nc.tensor.matmul(
                wpsum[:, r * 64 : (r + 1) * 64],
                warm[:],
                warm[:, 0:64],
                start=True,
                stop=True,
            )
        for r in range(6):
            nc.tensor.matmul(
                wpsum[:, (r % 4) * 128 : (r % 4) * 128 + 128],
                warm[:],
                warm[:],
                start=True,
                stop=True,
            )

        def emit_energy(icn, gi, halved=False):
            jb0, gsz = GROUPS[gi]
            et = epool.tile([128, G, IC], dt.float32, tag="et", name=f"et{icn}_{gi}")
            for g in range(gsz):
                jb = jb0 + g
                stat = xt_ch[jb // 4][:, (jb % 4) * 128 : (jb % 4 + 1) * 128]
                if halved:
                    # first group: start as soon as each w half-chunk lands
                    for h in range(2):
                        nc.tensor.matmul(
                            et[:, g, h * 256 : (h + 1) * 256],
                            stat,
                            w_ch[icn][:, h * 256 : (h + 1) * 256],
                            start=True,
                            stop=True,
                        )
                else:
                    nc.tensor.matmul(
                        et[:, g, :], stat, w_ch[icn][:], start=True, stop=True
                    )
            return et

        def emit_exp(icn, gi):
            gsz = GROUPS[gi][1]
            et = ets.pop((icn, gi))
            if gi in DVE_GROUPS:
                pt = pt16pool.tile(
                    [128, G, IC], dt.uint16, tag="pt16", name=f"pt16_{icn}_{gi}"
                )
                nc.vector.tensor_scalar(
                    out=pt[:, :gsz, :],
                    in0=et[:, :gsz, :],
                    scalar1=A16,
                    scalar2=B16,
                    op0=mybir.AluOpType.mult,
                    op1=mybir.AluOpType.add,
                )
                return ("u16", pt)
            pt = ptpool.tile(
                [128, G, IC], dt.bfloat16, tag="pt", name=f"pt{icn}_{gi}"
            )
            nc.scalar.activation(
                out=pt[:, :gsz, :],
                in_=et[:, :gsz, :],
                func=mybir.ActivationFunctionType.Exp,
                bias=shiftb[:],
            )
            return ("bf", pt)

        def alloc_oa(icn):
            oa_by_ic[icn] = [
                opool.tile([128, 129], dt.float32, tag=t, name=f"oa{t}_{icn}")
                for t in OA_TAGS
            ]

        def emit_attnv(icn, gi, kind_pt):
            kind, pt = kind_pt
            jb0, gsz = GROUPS[gi]
            oa = oa_by_ic[icn]
            for g in range(gsz):
                jb = jb0 + g
                for it in range(NIT):
                    st = pt[:, g, it * 128 : (it + 1) * 128]
                    if kind == "u16":
                        st = st.bitcast(dt.bfloat16)
                    nc.tensor.matmul(
                        oa[it][:],
                        st,
                        v_ch[jb // 4][:, jb % 4, 0:129],
                        start=(jb == 0),
                        stop=(jb == NJB - 1),
                    )

        def emit_blend(icn):
            oa = oa_by_ic.pop(icn)
            if icn == NIC - 1:
                # last chunk: read PSUM directly (no next attn@v needs the
                # banks) to shorten the tail
                rs = spool.tile([128, 4, 1], dt.float32, tag="rs", name=f"rs{icn}")
                for it in range(NIT):
                    nc.vector.reciprocal(rs[:, it, :], oa[it][:, 128:129])
                for it in range(NIT):
                    ti = icn * NIT + it
                    ot = outpool.tile(
                        [128, 128], dt.bfloat16, tag="ot", name=f"ot{ti}"
                    )
                    nc.vector.scalar_tensor_tensor(
                        out=ot[:],
                        in0=oa[it][:, 0:128],
                        scalar=rs[:, it, :],
                        in1=xr_ch[icn][:, it, :],
                        op0=mybir.AluOpType.mult,
                        op1=mybir.AluOpType.add,
                    )
                    eng = nc.sync if it % 2 == 0 else nc.scalar
                    eng.dma_start(out=out_d[ti], in_=ot[:])
                return
            osb = osb_pool.tile([128, 4, 129], dt.float32, tag="osb", name=f"osb{icn}")
            for it in range(NIT):
                nc.vector.tensor_copy(out=osb[:, it, :], in_=oa[it][:])
            rs = spool.tile([128, 4, 1], dt.float32, tag="rs", name=f"rs{icn}")
            nc.vector.reciprocal(rs[:], osb[:, :, 128:129])
            for it in range(NIT):
                ti = icn * NIT + it
                ot = outpool.tile([128, 128], dt.bfloat16, tag="ot", name=f"ot{ti}")
                nc.vector.scalar_tensor_tensor(
                    out=ot[:],
                    in0=osb[:, it, 0:128],
                    scalar=rs[:, it, :],
                    in1=xr_ch[icn][:, it, :],
                    op0=mybir.AluOpType.mult,
                    op1=mybir.AluOpType.add,
                )
                nc.sync.dma_start(out=out_d[ti], in_=ot[:])

        ets = {}
        oa_by_ic = {}

        # ---- uniform flat schedule: attn@v lags exp by one group so the PE
        # never waits on the freshest exp; 2-group energy lookahead ----
        flat = [(icn, gi) for icn in range(NIC) for gi in range(NG)]
        ets[flat[0]] = emit_energy(0, 0, halved=True)
        prev = None
        for fk, (icn, gi) in enumerate(flat):
            for ahead in (1, 2):
                if fk + ahead < len(flat) and flat[fk + ahead] not in ets:
                    ets[flat[fk + ahead]] = emit_energy(*flat[fk + ahead])
            if prev is not None:
                picn, pgi, ppt = prev
                if pgi == 0:
                    alloc_oa(picn)
                emit_attnv(picn, pgi, ppt)
                if pgi == NG - 1:
                    emit_blend(picn)
            prev = (icn, gi, emit_exp(icn, gi))
        picn, pgi, ppt = prev
        emit_attnv(picn, pgi, ppt)
        emit_blend(picn)

    nc.finalize()
    return nc


def get_nc():
    if "nc" not in _NC_CACHE:
        _NC_CACHE["nc"] = _build_nc()
    return _NC_CACHE["nc"]


def make_in_maps(x, Wq, Wk, Wv, gamma):
    import ml_dtypes

    x = np.asarray(x, dtype=np.float32)
    Wq = np.asarray(Wq, dtype=np.float32)
    Wk = np.asarray(Wk, dtype=np.float32)
    Wv = np.asarray(Wv, dtype=np.float32)
    gamma = np.asarray(gamma, dtype=np.float32)

    xf = x.reshape(B, N, C)
    A16 = (Wq @ Wk.T).astype(np.float16).astype(np.float32)
    wv16 = (gamma * Wv).astype(np.float16).astype(np.float32)

    in_maps = []
    for c in range(NCORES):
        b, ih = c // 2, c % 2
        x16 = xf[b].astype(np.float16)                 # [N, C]
        xT = x16.T                                     # [128, 4096]
        # rotate the j-order so this core's own i-rows are columns 0:NI
        # (softmax sums over j, so any j-order works as long as v matches)
        xTr = np.ascontiguousarray(np.roll(xT, -ih * NI, axis=1))
        sl = slice(ih * NI, (ih + 1) * NI)
        # w = A @ X_i^T over this core's own i-rows (fp16 storage)
        w = (A16 @ x16[sl].astype(np.float32).T).astype(np.float16)
        w = np.ascontiguousarray(w.reshape(128, NIC, 512).transpose(1, 0, 2))
        # v = X (gamma Wv), j-rotated to match xT, ones column baked at col 128
        vfull = x16.astype(np.float32) @ wv16          # [N, 128] fp32
        vrot = np.roll(vfull, -ih * NI, axis=0)
        vtile = np.zeros((N // 512, 128, 4, 132), np.float32)
        vtile[:, :, :, 0:128] = vrot.reshape(N // 512, 4, 128, 128).transpose(
            0, 2, 1, 3
        )
        vtile[:, :, :, 128] = 1.0
        xres = (
            xf[b][sl]
            .reshape(NIC, NIT, 128, C)
            .transpose(0, 2, 1, 3)
            .reshape(NIC, 128, NIT * 128)
        )
        in_maps.append(
            {
                "xT": xTr,
                "w": w,
                "v": np.ascontiguousarray(
                    vtile.reshape(N // 512, 128, 4 * 132)
                ).astype(ml_dtypes.bfloat16),
                "xres": np.ascontiguousarray(xres).astype(ml_dtypes.bfloat16),
            }
        )
    return in_maps


def assemble_out(results):
    outs = [
        np.asarray(results[c]["out"]).astype(np.float32).reshape(NI, C)
        for c in range(NCORES)
    ]
    full = np.stack(
        [np.concatenate([outs[2 * b], outs[2 * b + 1]], axis=0) for b in range(B)]
    )
    return full.reshape(B, Dd, Hh, Ww, C).astype(np.float32)


def kernel(x, Wq, Wk, Wv, gamma):
    from concourse.bass_utils import run_bass_kernel_spmd

    nc = get_nc()
    in_maps = make_in_maps(x, Wq, Wk, Wv, gamma)
    res = run_bass_kernel_spmd(nc, in_maps, core_ids=list(range(NCORES)))
    return assemble_out(res.results)
